# revision 1
# baseline (speedup 1.0000x reference)
"""Corr1d-x-group cost-volume kernel for Trainium2 (8 NeuronCores, SPMD).

Data-parallel over batch N=8: core i processes batch i.

Per core (inputs [16, 256, 512] f32 each, output [108, 256, 512] f32):
  out[g*27+ch, h, w] = 0.25 * sum_c f1[g*4+c, h, w] * f2[g*4+c, h, w+ch-23]
with zero padding outside w in [0, 512).

Implementation:
  - Inputs are DMA-cast f32->f16 on load (SWDGE cast DMA).
  - SBUF layout per 8-row h-block: partitions = (channel(16), h8(8)) = 128.
  - 27 shifted products on VectorE (fp16 tensor_tensor, 2x perf mode;
    dual parity copies of the padded f2 tile keep slices 4B-aligned).
  - Channel reduction (sum over c of each group g) via TensorE matmul with a
    constant block-diagonal 0.25 weight matrix [128, 32]; 4 shifts packed
    into one PSUM bank via tile_position column groups -> fp32 PSUM.
  - ScalarE copies PSUM->SBUF, HWDGE DMA stores to DRAM.
"""

import os
import numpy as np

import concourse.bass as bass
import concourse.bacc as bacc
import concourse.mybir as mybir
import concourse.tile as tile
from concourse import bass_utils

N, C, H, W = 8, 16, 256, 512
G = 4
TOP_CH = 27
RADIUS = 13
PAD_SHIFT = -10  # shift s = ch - 23 for ch in [0, 27)
OUT_CH = G * TOP_CH  # 108
HB = 32  # h rows per block; 4 channels * 32 rows = 128 partitions
NBLK = H // HB
PADL = 24  # f2 padded tile: column = w + PADL (even so slices align)
F2W = PADL + W + 8  # 544 columns, covers w in [-24, 520)

_CACHED = {}


def _reduction_weights() -> np.ndarray:
    # lhsT [K=(c, h32)=128, M=h32=32]: sums the 4 channels of a group and
    # applies the 1/sumelems scale.
    w = np.zeros((128, 32), np.float16)
    for c in range(G):
        for hh in range(HB):
            w[c * HB + hh, hh] = 0.25
    return w


def _build_program() -> bass.Bass:
    # Bacc (not raw Bass): its compile() splits multi-sem sync waits, which
    # TRN2 hardware limits to one per instruction.
    nc = bacc.Bacc(
        "TRN2",
        target_bir_lowering=False,
        debug=False,
        enable_asserts=False,
        num_devices=N,
    )
    f16 = mybir.dt.float16
    f32 = mybir.dt.float32

    l_in = nc.dram_tensor("l_in", [C, H, W], f32, kind="ExternalInput")
    r_in = nc.dram_tensor("r_in", [C, H, W], f32, kind="ExternalInput")
    w_red = nc.dram_tensor("w_red", [128, 32], f16, kind="ExternalInput")
    out = nc.dram_tensor("out", [OUT_CH, H, W], f32, kind="ExternalOutput")

    # Output viewed as [ch(27), g(4), h*w]: one shift's store for an h-block
    # is [1, 4, HB*W] -> a 2-dim AP against the [128, 512] SBUF stage tile
    # whose partition-major order is (g, h32, w).
    out_v = out.ap().rearrange("(g c) h w -> c g (h w)", g=G)

    with tile.TileContext(nc) as tc:
        with (
            tc.tile_pool(name="wpool", bufs=1) as wpool,
            tc.tile_pool(name="inpool", bufs=2) as inpool,
            tc.tile_pool(name="prodpool", bufs=4) as prodpool,
            tc.tile_pool(name="obpool", bufs=3) as obpool,
            tc.tile_pool(name="psumpool", bufs=2, space="PSUM") as psumpool,
        ):
            wt = wpool.tile([128, 32], f16)
            nc.sync.dma_start(wt[:], w_red[:])

            for ib in range(NBLK):
                h0 = ib * HB
                f1s = []
                f2es = []
                f2os = []
                for g in range(G):
                    f1 = inpool.tile([128, W], f16, tag=f"f1_{g}")
                    nc.gpsimd.dma_start(
                        f1[:], l_in[g * G : (g + 1) * G, h0 : h0 + HB, :]
                    )
                    f1s.append(f1)

                    f2e = inpool.tile([128, F2W], f16, tag=f"f2e_{g}")
                    nc.vector.memset(f2e[:, 0:PADL], 0.0)
                    nc.vector.memset(f2e[:, PADL + W : F2W], 0.0)
                    nc.gpsimd.dma_start(
                        f2e[:, PADL : PADL + W],
                        r_in[g * G : (g + 1) * G, h0 : h0 + HB, :],
                    )
                    f2es.append(f2e)
                    # Odd-parity tile: same data at column = w + (PADL-1), so
                    # odd shifts read from a 4B-aligned start. Loaded with its
                    # own cast-DMA (a DVE shift-copy trips the sync-wait cap).
                    f2o = inpool.tile([128, F2W], f16, tag=f"f2o_{g}")
                    nc.vector.memset(f2o[:, 0 : PADL - 1], 0.0)
                    nc.vector.memset(f2o[:, PADL - 1 + W : F2W], 0.0)
                    nc.gpsimd.dma_start(
                        f2o[:, PADL - 1 : PADL - 1 + W],
                        r_in[g * G : (g + 1) * G, h0 : h0 + HB, :],
                    )
                    f2os.append(f2o)

                for ch in range(TOP_CH):
                    col = PADL + ch - (RADIUS - PAD_SHIFT)  # PADL + shift
                    psumt = psumpool.tile([128, W], f32, tag="psumt")
                    for g in range(G):
                        if col % 2 == 0:
                            src = f2es[g][:, col : col + W]
                        else:
                            src = f2os[g][:, col - 1 : col - 1 + W]
                        p = prodpool.tile([128, W], f16, tag="prod")
                        nc.vector.tensor_mul(p[:], f1s[g][:], src)
                        nc.tensor.matmul(
                            psumt[32 * g : 32 * (g + 1), :],
                            wt[:],
                            p[:],
                            start=True,
                            stop=True,
                            tile_position=(0, 32 * g),
                        )
                    ob = obpool.tile([128, W], f32, tag="ob")
                    nc.scalar.copy(ob[:], psumt[:])
                    nc.sync.dma_start(
                        out_v[ch : ch + 1, :, h0 * W : (h0 + HB) * W],
                        ob[:],
                    )
    nc.compile()
    return nc


def kernel(l_in: np.ndarray, r_in: np.ndarray) -> np.ndarray:
    assert l_in.shape == (N, C, H, W) and r_in.shape == (N, C, H, W)
    l_in = np.ascontiguousarray(l_in, dtype=np.float32)
    r_in = np.ascontiguousarray(r_in, dtype=np.float32)

    if "nc" not in _CACHED:
        _CACHED["nc"] = _build_program()
    nc = _CACHED["nc"]

    w_np = _reduction_weights()
    in_maps = [
        {
            "l_in": np.ascontiguousarray(l_in[i]),
            "r_in": np.ascontiguousarray(r_in[i]),
            "w_red": w_np,
        }
        for i in range(N)
    ]
    trace = bool(int(os.environ.get("CORR_KERNEL_TRACE", "0")))
    kwargs = {}
    tdir = os.environ.get("CORR_KERNEL_TRACE_DIR")
    if trace and tdir:
        os.makedirs(tdir, exist_ok=True)
        kwargs["tmpdir"] = tdir
    res = bass_utils.run_bass_kernel_spmd(
        nc, in_maps, core_ids=list(range(N)), trace=trace, **kwargs
    )
    _CACHED["last_result"] = res
    return np.stack([res.results[i]["out"] for i in range(N)], axis=0)



# revision 5
# speedup vs baseline: 1.6642x; 1.6642x over previous
"""Corr1d-x-group cost-volume kernel for Trainium2 (8 NeuronCores, SPMD).

Data-parallel over batch N=8: core i processes batch i.

Per core (inputs [16, 256, 512] f32 each, output [108, 256, 512]):
  out[g*27+ch, h, w] = 0.25 * sum_c f1[g*4+c, h, w] * f2[g*4+c, h, w+ch-23]
with zero padding outside w in [0, 512).

v2 implementation (vs baseline):
  - Whole inputs resident in SBUF as f16, cast-loaded once (SWDGE f32->f16),
    pipelined in two j-halves; j-half loop is outermost so half-1 loads
    overlap half-0 compute.
  - Partition layout (hl8, c16): partition = (h//32)*16 + c, free =
    (j=h%32, w). One VectorE tensor_mul per (ch, half) with FD=8192 (3-dim
    strided APs) instead of 864 FD=512 muls -- amortizes per-op overhead.
  - Dual-parity padded f2 copies keep all DVE slices 4B-aligned (2x mode).
  - Channel reduction: TensorE matmul, K=128=(hl,c), M=32=(g,hl), one shared
    constant weight; 8 window-matmuls packed into a [128,1024] PSUM tile
    (2 banks) via tile_position, jj-major order so equal positions repeat
    back-to-back.
  - ScalarE evacuates PSUM f32 -> SBUF f16; HWDGE stores f16 (halves write
    traffic); host upcasts the gathered result to f32 (rel-err ~1e-3, well
    under the 2e-2 gate).
"""

import os
import numpy as np

import concourse.bass as bass
import concourse.bacc as bacc
import concourse.mybir as mybir
import concourse.tile as tile
from concourse import bass_utils

N, C, H, W = 8, 16, 256, 512
G = 4
TOP_CH = 27
RADIUS = 13
PAD_SHIFT = -10  # shift s = ch - 23 for ch in [0, 27)
OUT_CH = G * TOP_CH  # 108

HL = 8          # partition sub-index: h // 32
NJ = 32         # free rows per partition: j = h % 32
PADE = 24       # f2 even tile: value f2[w] at column 24 + w
PADO = 23       # f2 odd tile:  value f2[w] at column 23 + w
F2W = 544       # padded row width (even, so row strides stay 4B-aligned)

_CACHED = {}


def _reduction_weights() -> np.ndarray:
    # lhsT [K=(hl8,c16)=128, M=(g4,hl8)=32]: sums the 4 channels of each
    # group and applies the 1/sumelems scale.
    w = np.zeros((128, 32), np.float16)
    for c in range(C):
        for hl in range(HL):
            w[hl * C + c, (c // G) * HL + hl] = 0.25
    return w


def _build_program() -> bass.Bass:
    # Bacc (not raw Bass): its compile() splits multi-sem sync waits, which
    # TRN2 hardware limits to one per instruction.
    nc = bacc.Bacc(
        "TRN2",
        target_bir_lowering=False,
        debug=False,
        enable_asserts=False,
        num_devices=N,
    )
    f16 = mybir.dt.float16
    f32 = mybir.dt.float32

    l_in = nc.dram_tensor("l_in", [C, H, W], f32, kind="ExternalInput")
    r_in = nc.dram_tensor("r_in", [C, H, W], f32, kind="ExternalInput")
    w_red = nc.dram_tensor("w_red", [128, 32], f16, kind="ExternalInput")
    out = nc.dram_tensor("out", [OUT_CH, H, W], f16, kind="ExternalOutput")

    # h = hl*32 + j with j = wg*8 + jj*2 + u; output channel = g*27 + ch.
    # Stores are sliced per (ch, wg, jj): DRAM dims (g, hl, (u w)) match the
    # 32-partition PSUM block order m = g*8 + hl with free (u, w).
    out_v = out.ap().rearrange(
        "(g c) (hl wg jj u) w -> c wg jj g hl (u w)",
        g=G, c=TOP_CH, hl=HL, wg=4, jj=4, u=2,
    )

    with tile.TileContext(nc) as tc:
        with (
            tc.tile_pool(name="wpool", bufs=1) as wpool,
            tc.tile_pool(name="inpool", bufs=1) as inpool,
            tc.tile_pool(name="prodpool", bufs=3) as prodpool,
            tc.tile_pool(name="obpool", bufs=4) as obpool,
            tc.tile_pool(name="psumpool", bufs=4, space="PSUM") as psumpool,
        ):
            wt = wpool.tile([128, 32], f16)
            nc.sync.dma_start(wt[:], w_red[:])

            f1 = inpool.tile([128, NJ * W], f16)
            f2e = inpool.tile([128, NJ * F2W], f16)
            f2o = inpool.tile([128, NJ * F2W], f16)
            f1v = f1.rearrange("p (j w) -> p j w", w=W)
            f2ev = f2e.rearrange("p (j x) -> p j x", x=F2W)
            f2ov = f2o.rearrange("p (j x) -> p j x", x=F2W)

            # Static zero padding left/right of each 544-column row.
            nc.vector.memset(f2ev[:, :, 0:PADE], 0.0)
            nc.vector.memset(f2ev[:, :, PADE + W : F2W], 0.0)
            nc.vector.memset(f2ov[:, :, 0:PADO], 0.0)
            nc.vector.memset(f2ov[:, :, PADO + W : F2W], 0.0)

            # Cast-loads (SWDGE), per (j-half, hl): 16-partition slices whose
            # DRAM side is a plain [c, j, w] slice (<=3 dims after opt).
            def load_half(jh):
                j0 = 16 * jh
                for hl in range(HL):
                    p0 = hl * C
                    h0 = hl * NJ + j0
                    nc.gpsimd.dma_start(
                        f1v[p0 : p0 + C, j0 : j0 + 16, :],
                        l_in[:, h0 : h0 + 16, :],
                    )
                    nc.gpsimd.dma_start(
                        f2ev[p0 : p0 + C, j0 : j0 + 16, PADE : PADE + W],
                        r_in[:, h0 : h0 + 16, :],
                    )
                    nc.gpsimd.dma_start(
                        f2ov[p0 : p0 + C, j0 : j0 + 16, PADO : PADO + W],
                        r_in[:, h0 : h0 + 16, :],
                    )

            load_half(0)
            load_half(1)

            for jh in range(2):
                j0 = 16 * jh
                for ch in range(TOP_CH):
                    s = ch - (RADIUS - PAD_SHIFT)  # in [-23, 3]
                    if (PADE + s) % 2 == 0:
                        src3, col0 = f2ev, PADE + s
                    else:
                        src3, col0 = f2ov, PADO + s
                    p = prodpool.tile([128, 16 * W], f16, tag="prod")
                    p3 = p.rearrange("p (j w) -> p j w", w=W)
                    nc.vector.tensor_mul(
                        p3[:, :, :],
                        f1v[:, j0 : j0 + 16, :],
                        src3[:, j0 : j0 + 16, col0 : col0 + W],
                    )
                    # Two PSUM tiles per half (wg = jh*2 + wg_l); fill them
                    # jj-major so equal tile_positions are back-to-back.
                    psums = []
                    for wg_l in range(2):
                        psumt = psumpool.tile(
                            [128, 2 * W], f32, tag="ps", name="psumt"
                        )
                        psums.append(psumt)
                    for jj in range(4):
                        for wg_l in range(2):
                            for u in range(2):
                                jl = wg_l * 8 + jj * 2 + u
                                nc.tensor.matmul(
                                    psums[wg_l][
                                        32 * jj : 32 * (jj + 1),
                                        W * u : W * (u + 1),
                                    ],
                                    wt[:],
                                    p[:, W * jl : W * (jl + 1)],
                                    start=True,
                                    stop=True,
                                    tile_position=(0, 32 * jj),
                                )
                    for wg_l in range(2):
                        wg = jh * 2 + wg_l
                        ob = obpool.tile([128, 2 * W], f16, tag="ob")
                        nc.scalar.copy(ob[:], psums[wg_l][:])
                        for jj in range(4):
                            nc.sync.dma_start(
                                out_v[
                                    ch : ch + 1,
                                    wg : wg + 1,
                                    jj : jj + 1,
                                ],
                                ob[32 * jj : 32 * (jj + 1), :],
                            )
    nc.compile()
    return nc


def kernel(l_in: np.ndarray, r_in: np.ndarray) -> np.ndarray:
    assert l_in.shape == (N, C, H, W) and r_in.shape == (N, C, H, W)
    l_in = np.ascontiguousarray(l_in, dtype=np.float32)
    r_in = np.ascontiguousarray(r_in, dtype=np.float32)

    if "nc" not in _CACHED:
        _CACHED["nc"] = _build_program()
    nc = _CACHED["nc"]

    w_np = _reduction_weights()
    in_maps = [
        {
            "l_in": np.ascontiguousarray(l_in[i]),
            "r_in": np.ascontiguousarray(r_in[i]),
            "w_red": w_np,
        }
        for i in range(N)
    ]
    trace = bool(int(os.environ.get("CORR_KERNEL_TRACE", "0")))
    kwargs = {}
    tdir = os.environ.get("CORR_KERNEL_TRACE_DIR")
    if trace and tdir:
        os.makedirs(tdir, exist_ok=True)
        kwargs["tmpdir"] = tdir
    res = bass_utils.run_bass_kernel_spmd(
        nc, in_maps, core_ids=list(range(N)), trace=trace, **kwargs
    )
    _CACHED["last_result"] = res
    return np.stack(
        [res.results[i]["out"] for i in range(N)], axis=0
    ).astype(np.float32)


# revision 6
# speedup vs baseline: 1.9003x; 1.1418x over previous
"""Corr1d-x-group cost-volume kernel for Trainium2 (8 NeuronCores, SPMD).

Data-parallel over batch N=8: core i processes batch i.

Per core (inputs [16, 256, 512] f32 each, output [108, 256, 512]):
  out[g*27+ch, h, w] = 0.25 * sum_c f1[g*4+c, h, w] * f2[g*4+c, h, w+ch-23]
with zero padding outside w in [0, 512).

v3 implementation:
  - Whole inputs resident in SBUF as f16 (cast-loaded once via SWDGE).
    Partition layout (c16, hl8): partition = c*8 + h//32, free = (j=h%32, w).
  - One VectorE tensor_mul per (ch, j-half) with FD=8192 (3-dim strided APs):
    54 muls total; dual-parity padded f2 keeps slices 4B-aligned (2x mode).
    The odd-parity copy is built on-chip by ScalarE from the even copy
    (saves an 8 MB HBM re-read); odd-shift channels are processed after
    even-shift ones so the copy hides behind compute.
  - Channel reduction: TensorE matmul, K=128=(c,hl), M=32=(g,hl), shared
    constant weight; 16 window-matmuls (jj4 x u4) pack one [128,2048] PSUM
    tile (4 banks) via tile_position + bank-aligned free offsets.
  - One ScalarE evac per chunk (PSUM f32 -> SBUF f16), 4 HWDGE stores per
    chunk ([32,2048] slices, 4KB DRAM runs). f16 DRAM output; host upcasts
    (rel-err ~1e-3, well under the 2e-2 gate).
"""

import os
import numpy as np

import concourse.bass as bass
import concourse.bacc as bacc
import concourse.mybir as mybir
import concourse.tile as tile
from concourse import bass_utils

N, C, H, W = 8, 16, 256, 512
G = 4
TOP_CH = 27
RADIUS = 13
PAD_SHIFT = -10  # shift s = ch - 23 for ch in [0, 27)
OUT_CH = G * TOP_CH  # 108

HL = 8          # partition sub-index: h // 32
NJ = 32         # free rows per partition: j = h % 32
PADE = 24       # f2 even tile: value f2[w] at column 24 + w
PADO = 23       # f2 odd tile:  value f2[w] at column 23 + w
F2W = 544       # padded row width (even, so row strides stay 4B-aligned)

_CACHED = {}


def _reduction_weights() -> np.ndarray:
    # lhsT [K=(c16,hl8)=128, M=(g4,hl8)=32]: sums the 4 channels of each
    # group and applies the 1/sumelems scale.
    w = np.zeros((128, 32), np.float16)
    for c in range(C):
        for hl in range(HL):
            w[c * HL + hl, (c // G) * HL + hl] = 0.25
    return w


def _build_program() -> bass.Bass:
    # Bacc (not raw Bass): its compile() splits multi-sem sync waits, which
    # TRN2 hardware limits to one per instruction.
    nc = bacc.Bacc(
        "TRN2",
        target_bir_lowering=False,
        debug=False,
        enable_asserts=False,
        num_devices=N,
    )
    f16 = mybir.dt.float16
    f32 = mybir.dt.float32

    l_in = nc.dram_tensor("l_in", [C, H, W], f32, kind="ExternalInput")
    r_in = nc.dram_tensor("r_in", [C, H, W], f32, kind="ExternalInput")
    w_red = nc.dram_tensor("w_red", [128, 32], f16, kind="ExternalInput")
    out = nc.dram_tensor("out", [OUT_CH, H, W], f16, kind="ExternalOutput")

    # h = hl*32 + wg*16 + jj*4 + u; output channel = g*27 + ch.
    # Stores are sliced per (ch, wg, jj): DRAM dims (g, hl, (u w)) match the
    # 32-partition PSUM block order m = g*8 + hl with free (u, w).
    out_v = out.ap().rearrange(
        "(g c) (hl wg jj u) w -> c wg jj g hl (u w)",
        g=G, c=TOP_CH, hl=HL, wg=2, jj=4, u=4,
    )
    # f1 loads: one [128, 8192] cast-DMA per j-half.
    l_src = l_in.ap().rearrange("c (hl j) w -> (c hl) j w", hl=HL)
    # f2 loads: per-channel slices (8 partitions each) keep DRAM APs <=3 dims.
    r_src = r_in.ap().rearrange("c (hl j) w -> c hl j w", hl=HL)

    # Process even shifts (f2e) first so the on-chip f2o build (ScalarE)
    # hides behind compute. col0 = PADE + s must be even for f2e.
    ch_even_par = [ch for ch in range(TOP_CH) if (PADE + ch - 23) % 2 == 0]
    ch_odd_par = [ch for ch in range(TOP_CH) if (PADE + ch - 23) % 2 == 1]
    ch_order = ch_even_par + ch_odd_par

    with tile.TileContext(nc) as tc:
        with (
            tc.tile_pool(name="wpool", bufs=1) as wpool,
            tc.tile_pool(name="inpool", bufs=1) as inpool,
            tc.tile_pool(name="prodpool", bufs=3) as prodpool,
            tc.tile_pool(name="obpool", bufs=3) as obpool,
            tc.tile_pool(name="psumpool", bufs=2, space="PSUM") as psumpool,
        ):
            wt = wpool.tile([128, 32], f16)
            nc.sync.dma_start(wt[:], w_red[:])

            f1 = inpool.tile([128, NJ * W], f16)
            f2e = inpool.tile([128, NJ * F2W], f16)
            f2o = inpool.tile([128, NJ * F2W], f16)
            f1v = f1.rearrange("p (j w) -> p j w", w=W)
            f2ev = f2e.rearrange("p (j x) -> p j x", x=F2W)
            f2ov = f2o.rearrange("p (j x) -> p j x", x=F2W)

            # Static zero padding left/right of each 544-column row (f2o
            # inherits its pads from the shifted f2e copy).
            nc.vector.memset(f2ev[:, :, 0:PADE], 0.0)
            nc.vector.memset(f2ev[:, :, PADE + W : F2W], 0.0)

            for jh in range(2):
                j0 = 16 * jh
                nc.gpsimd.dma_start(
                    f1v[:, j0 : j0 + 16, :], l_src[:, j0 : j0 + 16, :]
                )
                for c in range(C):
                    nc.gpsimd.dma_start(
                        f2ev[c * HL : (c + 1) * HL, j0 : j0 + 16, PADE : PADE + W],
                        r_src[c, :, j0 : j0 + 16, :],
                    )
                # Odd-parity copy: f2o[., j, x] = f2e[., j, x+1]  (= f2[x-23]).
                # Column 543 is never read by any shift; leave it.
                nc.scalar.copy(
                    f2ov[:, j0 : j0 + 16, 0 : F2W - 1],
                    f2ev[:, j0 : j0 + 16, 1:F2W],
                )

            for jh in range(2):
                j0 = 16 * jh
                for ch in ch_order:
                    s = ch - (RADIUS - PAD_SHIFT)  # in [-23, 3]
                    if (PADE + s) % 2 == 0:
                        src3, col0 = f2ev, PADE + s
                    else:
                        src3, col0 = f2ov, PADO + s
                    p = prodpool.tile([128, 16 * W], f16, tag="prod")
                    p3 = p.rearrange("p (j w) -> p j w", w=W)
                    nc.vector.tensor_mul(
                        p3[:, :, :],
                        f1v[:, j0 : j0 + 16, :],
                        src3[:, j0 : j0 + 16, col0 : col0 + W],
                    )
                    # 16 windows -> one [128, 2048] PSUM tile (4 banks):
                    # partition block jj via tile_position, bank u via the
                    # free offset. jj-major so equal positions repeat.
                    psumt = psumpool.tile([128, 4 * W], f32, tag="ps")
                    for jj in range(4):
                        for u in range(4):
                            jl = jj * 4 + u
                            nc.tensor.matmul(
                                psumt[
                                    32 * jj : 32 * (jj + 1),
                                    W * u : W * (u + 1),
                                ],
                                wt[:],
                                p[:, W * jl : W * (jl + 1)],
                                start=True,
                                stop=True,
                                tile_position=(0, 32 * jj),
                            )
                    ob = obpool.tile([128, 4 * W], f16, tag="ob")
                    nc.scalar.copy(ob[:], psumt[:])
                    for jj in range(4):
                        nc.sync.dma_start(
                            out_v[ch : ch + 1, jh : jh + 1, jj : jj + 1],
                            ob[32 * jj : 32 * (jj + 1), :],
                        )
    nc.compile()
    return nc


def kernel(l_in: np.ndarray, r_in: np.ndarray) -> np.ndarray:
    assert l_in.shape == (N, C, H, W) and r_in.shape == (N, C, H, W)
    l_in = np.ascontiguousarray(l_in, dtype=np.float32)
    r_in = np.ascontiguousarray(r_in, dtype=np.float32)

    if "nc" not in _CACHED:
        _CACHED["nc"] = _build_program()
    nc = _CACHED["nc"]

    w_np = _reduction_weights()
    in_maps = [
        {
            "l_in": np.ascontiguousarray(l_in[i]),
            "r_in": np.ascontiguousarray(r_in[i]),
            "w_red": w_np,
        }
        for i in range(N)
    ]
    trace = bool(int(os.environ.get("CORR_KERNEL_TRACE", "0")))
    kwargs = {}
    tdir = os.environ.get("CORR_KERNEL_TRACE_DIR")
    if trace and tdir:
        os.makedirs(tdir, exist_ok=True)
        kwargs["tmpdir"] = tdir
    res = bass_utils.run_bass_kernel_spmd(
        nc, in_maps, core_ids=list(range(N)), trace=trace, **kwargs
    )
    _CACHED["last_result"] = res
    return np.stack(
        [res.results[i]["out"] for i in range(N)], axis=0
    ).astype(np.float32)


# revision 7
# speedup vs baseline: 2.3925x; 1.2590x over previous
"""Corr1d-x-group cost-volume kernel for Trainium2 (8 NeuronCores, SPMD).

Data-parallel over batch N=8: core i processes batch i.

Per core (inputs [16, 256, 512] f32 each, output [108, 256, 512]):
  out[g*27+ch, h, w] = 0.25 * sum_c f1[g*4+c, h, w] * f2[g*4+c, h, w+ch-23]
with zero padding outside w in [0, 512).

v4 implementation:
  - Partition layout (c16, hl8): partition = c*8 + h//32, free = (j=h%32, w).
    f1 and a tight f2 copy are cast-loaded f32->f16 with 4 big SWDGE DMAs
    (one per tensor per j-half); ScalarE then builds two padded f2 copies
    on-chip (even parity at column 24+w, odd parity at 23+w) so every DVE
    slice start stays 4B-aligned (2x tensor_tensor mode). The tight f2
    staging tiles borrow product-pool slots (dead after the builds).
  - One VectorE tensor_mul per (ch, j-half), FD=8192: 54 muls total.
    Channels using the even-parity copy run first so the odd-parity build
    hides behind compute.
  - Channel reduction: TensorE matmul, K=128=(c,hl), M=32=(g,hl), shared
    constant weight; 16 window-matmuls (jj4 x u4) pack one [128,2048] PSUM
    tile (4 banks) via tile_position + bank-aligned free offsets.
  - One ScalarE evac per chunk (PSUM f32 -> SBUF f16) and ONE contiguous
    512KB HWDGE store per chunk into a chunk-major DRAM tensor
    [54, 128, 2048]; the host undoes the permutation during the f16->f32
    upcast it performs anyway. (rel-err ~1e-3, well under the 2e-2 gate.)
"""

import os
import numpy as np

import concourse.bass as bass
import concourse.bacc as bacc
import concourse.mybir as mybir
import concourse.tile as tile
from concourse import bass_utils

N, C, H, W = 8, 16, 256, 512
G = 4
TOP_CH = 27
RADIUS = 13
PAD_SHIFT = -10  # shift s = ch - 23 for ch in [0, 27)
OUT_CH = G * TOP_CH  # 108

HL = 8          # partition sub-index: h // 32
NJ = 32         # free rows per partition: j = h % 32
PADE = 24       # f2 even tile: value f2[w] at column 24 + w
PADO = 23       # f2 odd tile:  value f2[w] at column 23 + w
F2W = 544       # padded row width (even, so row strides stay 4B-aligned)
NCHUNK = TOP_CH * 2  # (ch, j-half) chunks

_CACHED = {}


def _reduction_weights() -> np.ndarray:
    # lhsT [K=(c16,hl8)=128, M=(g4,hl8)=32]: sums the 4 channels of each
    # group and applies the 1/sumelems scale.
    w = np.zeros((128, 32), np.float16)
    for c in range(C):
        for hl in range(HL):
            w[c * HL + hl, (c // G) * HL + hl] = 0.25
    return w


def _build_program() -> bass.Bass:
    # Bacc (not raw Bass): its compile() splits multi-sem sync waits, which
    # TRN2 hardware limits to one per instruction.
    nc = bacc.Bacc(
        "TRN2",
        target_bir_lowering=False,
        debug=False,
        enable_asserts=False,
        num_devices=N,
    )
    f16 = mybir.dt.float16
    f32 = mybir.dt.float32

    l_in = nc.dram_tensor("l_in", [C, H, W], f32, kind="ExternalInput")
    r_in = nc.dram_tensor("r_in", [C, H, W], f32, kind="ExternalInput")
    w_red = nc.dram_tensor("w_red", [128, 32], f16, kind="ExternalInput")
    # Chunk-major output: [chunk=(ch,jh), partition=(jj,g,hl), (u,w)].
    out = nc.dram_tensor("out", [NCHUNK, 128, 4 * W], f16, kind="ExternalOutput")

    l_src = l_in.ap().rearrange("c (hl j) w -> (c hl) j w", hl=HL)
    r_src = r_in.ap().rearrange("c (hl j) w -> (c hl) j w", hl=HL)

    # Process even-parity shifts (f2e) first so the on-chip f2o build
    # (ScalarE) hides behind compute. col0 = PADE + s must be even for f2e.
    ch_even_par = [ch for ch in range(TOP_CH) if (PADE + ch - 23) % 2 == 0]
    ch_odd_par = [ch for ch in range(TOP_CH) if (PADE + ch - 23) % 2 == 1]
    ch_order = ch_even_par + ch_odd_par

    with tile.TileContext(nc) as tc:
        with (
            tc.tile_pool(name="wpool", bufs=1) as wpool,
            tc.tile_pool(name="inpool", bufs=1) as inpool,
            tc.tile_pool(name="prodpool", bufs=4) as prodpool,
            tc.tile_pool(name="obpool", bufs=3) as obpool,
            tc.tile_pool(name="psumpool", bufs=2, space="PSUM") as psumpool,
        ):
            wt = wpool.tile([128, 32], f16)
            nc.sync.dma_start(wt[:], w_red[:])

            f1 = inpool.tile([128, NJ * W], f16)
            f2e = inpool.tile([128, NJ * F2W], f16)
            f2o = inpool.tile([128, NJ * F2W], f16)
            f1v = f1.rearrange("p (j w) -> p j w", w=W)
            f2ev = f2e.rearrange("p (j x) -> p j x", x=F2W)
            f2ov = f2o.rearrange("p (j x) -> p j x", x=F2W)

            # Static zero padding left/right of each 544-column row.
            nc.vector.memset(f2ev[:, :, 0:PADE], 0.0)
            nc.vector.memset(f2ev[:, :, PADE + W : F2W], 0.0)
            nc.vector.memset(f2ov[:, :, 0:PADO], 0.0)
            nc.vector.memset(f2ov[:, :, PADO + W : F2W], 0.0)

            for jh in range(2):
                j0 = 16 * jh
                nc.gpsimd.dma_start(
                    f1v[:, j0 : j0 + 16, :], l_src[:, j0 : j0 + 16, :]
                )
                # Tight f2 staging tile borrows a product-pool slot.
                f2t = prodpool.tile([128, 16 * W], f16, tag="prod", name="f2t")
                f2tv = f2t.rearrange("p (j w) -> p j w", w=W)
                nc.gpsimd.dma_start(f2tv[:, :, :], r_src[:, j0 : j0 + 16, :])
                # Padded parity copies (ScalarE): even is 4B-aligned (fast
                # mode), odd is the slow 1x copy but hides behind compute.
                nc.scalar.copy(
                    f2ev[:, j0 : j0 + 16, PADE : PADE + W], f2tv[:, :, :]
                )
                nc.scalar.copy(
                    f2ov[:, j0 : j0 + 16, PADO : PADO + W], f2tv[:, :, :]
                )

            for jh in range(2):
                j0 = 16 * jh
                for ch in ch_order:
                    s = ch - (RADIUS - PAD_SHIFT)  # in [-23, 3]
                    if (PADE + s) % 2 == 0:
                        src3, col0 = f2ev, PADE + s
                    else:
                        src3, col0 = f2ov, PADO + s
                    p = prodpool.tile([128, 16 * W], f16, tag="prod", name="p")
                    p3 = p.rearrange("p (j w) -> p j w", w=W)
                    nc.vector.tensor_mul(
                        p3[:, :, :],
                        f1v[:, j0 : j0 + 16, :],
                        src3[:, j0 : j0 + 16, col0 : col0 + W],
                    )
                    # 16 windows -> one [128, 2048] PSUM tile (4 banks):
                    # partition block jj via tile_position, bank u via the
                    # free offset. jj-major so equal positions repeat.
                    psumt = psumpool.tile([128, 4 * W], f32, tag="ps", name="ps")
                    for jj in range(4):
                        for u in range(4):
                            jl = jj * 4 + u
                            nc.tensor.matmul(
                                psumt[
                                    32 * jj : 32 * (jj + 1),
                                    W * u : W * (u + 1),
                                ],
                                wt[:],
                                p[:, W * jl : W * (jl + 1)],
                                start=True,
                                stop=True,
                                tile_position=(0, 32 * jj),
                            )
                    ob = obpool.tile([128, 4 * W], f16, tag="ob", name="ob")
                    nc.scalar.copy(ob[:], psumt[:])
                    nc.sync.dma_start(
                        out.ap()[ch * 2 + jh : ch * 2 + jh + 1], ob[:]
                    )
    nc.compile()
    return nc


def kernel(l_in: np.ndarray, r_in: np.ndarray) -> np.ndarray:
    assert l_in.shape == (N, C, H, W) and r_in.shape == (N, C, H, W)
    l_in = np.ascontiguousarray(l_in, dtype=np.float32)
    r_in = np.ascontiguousarray(r_in, dtype=np.float32)

    if "nc" not in _CACHED:
        _CACHED["nc"] = _build_program()
    nc = _CACHED["nc"]

    w_np = _reduction_weights()
    in_maps = [
        {
            "l_in": np.ascontiguousarray(l_in[i]),
            "r_in": np.ascontiguousarray(r_in[i]),
            "w_red": w_np,
        }
        for i in range(N)
    ]
    trace = bool(int(os.environ.get("CORR_KERNEL_TRACE", "0")))
    kwargs = {}
    tdir = os.environ.get("CORR_KERNEL_TRACE_DIR")
    if trace and tdir:
        os.makedirs(tdir, exist_ok=True)
        kwargs["tmpdir"] = tdir
    res = bass_utils.run_bass_kernel_spmd(
        nc, in_maps, core_ids=list(range(N)), trace=trace, **kwargs
    )
    _CACHED["last_result"] = res

    # Undo the device's chunk-major layout while upcasting to f32:
    # out_d[ch, jh, jj, g, hl, u, w] -> out[g*27+ch, hl*32+jh*16+jj*4+u, w].
    outs = []
    for i in range(N):
        x = res.results[i]["out"].reshape(TOP_CH, 2, 4, G, HL, 4, W)
        x = x.transpose(3, 0, 4, 1, 2, 5, 6).reshape(OUT_CH, H, W)
        outs.append(x)
    return np.stack(outs, axis=0).astype(np.float32)
